# revision 15
# baseline (speedup 1.0000x reference)
"""Trainium2 Bass kernel for nn_BatchRankingMSE_Loss (N=8192, 8 cores).

Label-sorted reformulation (margin M=2, eps=1e-4):
  Sort positions by label (host argsort). With q = preds[perm], every pair
  a<b has sign(l_b - l_a) = +1 (ties corrected on host), so with
  z = M + q_a - q_b and u = 1{z > 0}:
    ranking = sum_{a<b} relu(z) = sum_{a<b} z*u
            = sum_a (M+q_a)*rowsum_u[a] + sum_b (-q_b)*colsum_u[b]
    grad_a  = rowsum_u[a] - colsum_u[a]
  So the device only needs the indicator u and its row/col sums - no relu
  pass, no sign pass, no multiplies, and the pair count is halved.

Uniform SPMD sharding of the strict upper triangle:
  Core g owns row-tiles at rows rs = 128g + 1024i (slot i = 0..7, 128 rows
  each). Its column input is the SHIFTED array Pc[j] = -q[j + 128g] (fp16)
  padded with sentinel -30000 so slot i's big span is always
  Pc[1024i+128 : 8192] - core-independent extents; sentinel columns yield
  exactly u = 0. Diag block of slot i is Pc[1024i : 1024i+128] with a
  threshold tile thd that folds in the strict b>a mask.

Per slot i (tiles [128 partition rows x F free cols], fp16):
  u big span:  DVE ts is_gt + add-reduce (some slots)
               ACT Sigmoid(65536*z) + native accumulator (other slots)
               accum_out = rowsum_u
  colsum(u) over partitions: TensorE onehot-matmuls into one PSUM bank
               [16 slabs x 512], prezeroed, accumulated across slots
  diag: u via DVE stt is_gt(thd) (mask folded in)
Host folds rowsums/colsums into ranking/grad-norm + tie correction.
"""

import numpy as np
import ml_dtypes

MARGIN = 2.0
EPS = 1e-4
N = 8192
NCORES = 8
NSLOT = 8
SENT = -192.0
SIGK = 65536.0

# engine owning each slot's u pass (extents E_i = 8064 - 1024*i)
U_ENGINE = {0: "act", 1: "dve", 2: "dve", 3: "act", 4: "act",
            5: "act", 6: "dve", 7: "dve"}
DVE_SLOTS = [i for i, e in U_ENGINE.items() if e == "dve"]
ACT_SLOTS = [i for i, e in U_ENGINE.items() if e == "act"]


def _make_subs():
    """Flat sub-pass list: (slot, lo, hi, eng, accum_col)."""
    subs = []
    col = 0
    for i in range(NSLOT):
        c0 = 1024 * i + 128
        eng = U_ENGINE[i]
        step = 4096 if eng == "dve" else 2048
        lo = c0
        while lo < N:
            hi = min(lo + step, N)
            e = eng
            # fine balance: slot 5's tail sub runs on DVE
            if i == 5 and lo > c0:
                e = "dve"
            subs.append((i, lo, hi, e, col))
            col += 1
            lo = hi
    return subs


SUBS = _make_subs()
NSUB = len(SUBS)
# accum layout: cols 0..NSUB-1 per sub, NSUB..NSUB+7 diag, NSUB+8 = mse
NACC = NSUB + NSLOT + 1

_CACHE = {}
LAST_RESULTS = None


def build_nc():
    import concourse.bass as bass
    import concourse.mybir as mybir
    from concourse import bacc, tile

    dt = mybir.dt
    Af = mybir.ActivationFunctionType
    Op = mybir.AluOpType

    nc = bacc.Bacc(None)
    pc_in = nc.dram_tensor("pc", [128, N], dt.float8e4, kind="ExternalInput")
    # f32 smalls: cols 0-7 sa=(M+q_a), 8-15 sb=-(M+q_a), 16-23 sac=SIGK*sa,
    #             24-31 prow, 32-39 lrow
    f32s_in = nc.dram_tensor("f32s", [128, 40], dt.float32,
                             kind="ExternalInput")
    # fp16 smalls: cols 0-1023 thd (diag thresholds+mask),
    #              1024-1295 onehot lhsT variants (17 x 16, #16 = zeros)
    f16s_in = nc.dram_tensor("f16s", [128, 1296], dt.float16,
                             kind="ExternalInput")

    uacc_out = nc.dram_tensor("uacc", [128, NACC], dt.float32,
                              kind="ExternalOutput")
    gcol_out = nc.dram_tensor("gcol", [16, 512], dt.float32,
                              kind="ExternalOutput")

    slot_max = {i: N - (1024 * i + 128) for i in range(NSLOT)}
    dve_max = max(slot_max[i] for i in DVE_SLOTS)
    act_max = max(slot_max[i] for i in ACT_SLOTS)

    with tile.TileContext(nc) as tc:
        with (
            tc.tile_pool(name="persist", bufs=1) as pp,
            tc.tile_pool(name="udve", bufs=3) as up_d,
            tc.tile_pool(name="uact", bufs=3) as up_a,
            tc.tile_pool(name="dwork", bufs=3) as wp,
            tc.tile_pool(name="psum", bufs=1, space="PSUM") as qp,
        ):
            pc = pp.tile([128, N], dt.float8e4)
            f32s = pp.tile([128, 40], dt.float32)
            f16s = pp.tile([128, 1296], dt.float16)
            dmse = pp.tile([128, 8], dt.float32)
            sqms = pp.tile([128, 8], dt.float32)
            # one accum tile; engines write disjoint column ranges
            uacc_all = pp.tile([128, NACC], dt.float32)
            gsb = pp.tile([16, 512], dt.float32)

            gb = qp.tile([16, 512], dt.float32, tag="gb", name="gb")

            sa = f32s[:, 0:8]
            sb = f32s[:, 8:16]
            sac = f32s[:, 16:24]
            pr = f32s[:, 24:32]
            lr = f32s[:, 32:40]
            thd = f16s[:, 0:1024]

            def oneh(v):
                return f16s[:, 1024 + 16 * v:1024 + 16 * (v + 1)]

            # input DMAs. pc is host-replicated [128, N] so each transfer
            # is ~128 descriptors (vs 128/chunk for partition_broadcast);
            # DGE dispatch ~14-40ns/descriptor dominates, so use 3 queues.
            nc.sync.dma_start(pc[:, 0:2688], pc_in[:, 0:2688])
            nc.scalar.dma_start(pc[:, 2688:5376], pc_in[:, 2688:5376])
            nc.gpsimd.dma_start(pc[:, 5376:N], pc_in[:, 5376:N])
            nc.sync.dma_start(f32s[:], f32s_in[:])
            nc.gpsimd.dma_start(f16s[:], f16s_in[:])

            # pre-load the sigmoid table while DMAs are in flight
            warm = pp.tile([128, 1], dt.float16)
            nc.scalar.activation(warm[:], f32s[:, 0:1], Af.Sigmoid,
                                 bias=0.0, scale=1.0)

            # prezero the PSUM colsum bank with a zero-weights matmul
            nc.tensor.matmul(gb[:], oneh(16), f16s[:, 0:512],
                             start=True, stop=False, skip_group_check=True)

            # mse partials: sum_free (p-l)^2 per partition
            nc.vector.scalar_tensor_tensor(
                dmse[:], pr, 0.0, lr, op0=Op.add, op1=Op.subtract)
            nc.vector.scalar_tensor_tensor(
                sqms[:], dmse[:], 1.0, dmse[:], op0=Op.mult, op1=Op.mult,
                accum_out=uacc_all[:, NSUB + NSLOT:NSUB + NSLOT + 1])

            u_tiles = {}
            cur_slot = -1
            for (i, lo, hi_s, eng, col) in SUBS:
                c0 = 1024 * i + 128
                if i != cur_slot:
                    cur_slot = i
                    # --- diag block (Pc cols [1024i, 1024i+128)) ---
                    ud = wp.tile([128, 128], dt.float16, tag="ud")
                    ds = slice(1024 * i, 1024 * i + 128)
                    nc.vector.scalar_tensor_tensor(
                        ud[:], pc[:, ds], 0.0, thd[:, 128 * i:128 * (i + 1)],
                        op0=Op.add, op1=Op.is_gt,
                        accum_out=uacc_all[:, NSUB + i:NSUB + i + 1])
                    nc.tensor.matmul(gb[:, 0:128], oneh(2 * i), ud[:],
                                     start=False, stop=False,
                                     skip_group_check=True)
                    if U_ENGINE[i] == "dve":
                        u_t = up_d.tile([128, dve_max], dt.float16, tag="u_d")
                    else:
                        u_t = up_a.tile([128, act_max], dt.float16, tag="u_a")
                # --- big span u sub-pass + rowsum accum ---
                if eng == "dve":
                    # ts-reduce: out = in0 op0 s1; accum = reduce(out, op1)
                    nc.vector.tensor_scalar(
                        u_t[:, lo - c0:hi_s - c0], pc[:, lo:hi_s],
                        sb[:, i:i + 1], 0.0,
                        op0=Op.is_gt, op1=Op.add,
                        accum_out=uacc_all[:, col:col + 1])
                else:
                    nc.scalar.activation(
                        u_t[:, lo - c0:hi_s - c0], pc[:, lo:hi_s],
                        Af.Sigmoid, bias=sac[:, i:i + 1], scale=SIGK,
                        accum_out=uacc_all[:, col:col + 1])
                # PE colsums for this sub-range (512 grid)
                off = lo
                while off < hi_s:
                    s = off // 512
                    hi = min((s + 1) * 512, hi_s)
                    nc.tensor.matmul(
                        gb[:, off - 512 * s:hi - 512 * s], oneh(s),
                        u_t[:, off - c0:hi - c0],
                        start=False, stop=False, skip_group_check=True)
                    off = hi

            nc.vector.tensor_copy(gsb[:], gb[:])
            nc.sync.dma_start(gcol_out[:], gsb[:])
            nc.gpsimd.dma_start(uacc_out[:], uacc_all[:])
    if not nc.is_finalized():
        nc.finalize()
    return nc


def make_in_maps(preds, labels, ncores=NCORES):
    preds = np.asarray(preds, dtype=np.float32)
    labels = np.asarray(labels, dtype=np.float32)
    perm = np.argsort(labels, kind="stable")
    q = preds[perm].astype(np.float64)

    onehots = np.zeros((128, 272), dtype=np.float16)
    for v in range(16):
        onehots[:, 16 * v + v] = 1.0

    in_maps = []
    for g in range(ncores):
        sh = 128 * g
        pcv = np.full(N, SENT, dtype=np.float64)
        pcv[:N - sh] = -q[sh:]
        qa = np.empty((128, NSLOT), dtype=np.float64)
        for i in range(NSLOT):
            qa[:, i] = q[sh + 1024 * i: sh + 1024 * i + 128]
        rows = slice(g * 1024, (g + 1) * 1024)
        f32s = np.empty((128, 40), dtype=np.float32)
        f32s[:, 0:8] = MARGIN + qa
        f32s[:, 8:16] = -(MARGIN + qa)
        f32s[:, 16:24] = SIGK * (MARGIN + qa)
        f32s[:, 24:32] = preds[rows].reshape(8, 128).T
        f32s[:, 32:40] = labels[rows].reshape(8, 128).T
        f16s = np.empty((128, 1296), dtype=np.float16)
        jj = np.arange(128)
        for i in range(NSLOT):
            f16s[:, 128 * i:128 * (i + 1)] = np.where(
                jj[None, :] > jj[:, None],
                (-(MARGIN + qa[:, i]))[:, None], 30000.0)
        f16s[:, 1024:1296] = onehots
        pc8 = np.broadcast_to(pcv.astype(ml_dtypes.float8_e4m3),
                              (128, N)).copy()
        in_maps.append({
            "pc": pc8,
            "f32s": f32s,
            "f16s": f16s,
        })
    return in_maps


def combine(results, preds, labels):
    preds = np.asarray(preds, dtype=np.float32)
    labels = np.asarray(labels, dtype=np.float32)
    perm = np.argsort(labels, kind="stable")
    q = preds[perm].astype(np.float64)
    ls = labels[perm]

    t_total = 0.0
    rowsum = np.zeros(N)
    colsum = np.zeros(N)
    msesum = 0.0
    for g, res in enumerate(results):
        sh = 128 * g
        ua = res["uacc"].astype(np.float64)
        pcv = np.full(N, SENT, dtype=np.float64)
        pcv[:N - sh] = -q[sh:].astype(np.float32).astype(
            ml_dtypes.float8_e4m3).astype(np.float64)
        slot_cols = {i: [] for i in range(NSLOT)}
        for (i, lo, hi, eng, col) in SUBS:
            slot_cols[i].append(col)
        for i in range(NSLOT):
            rows = slice(sh + 1024 * i, sh + 1024 * i + 128)
            rs_i = ua[:, slot_cols[i]].sum(1) + ua[:, NSUB + i]
            rowsum[rows] += rs_i
            # ranking row-part: sum_a (M+q_a) * rowsum_u[a]
            t_total += ((MARGIN + q[rows]) * rs_i).sum()
        gc = res["gcol"].astype(np.float64).reshape(-1)
        # ranking col-part: sum_b (-q_b) * colsum_u[b] (device fp16 pc vals;
        # sentinel cols excluded - their colsums are ~0 but pcv is huge)
        t_total += (pcv[:N - sh] * gc[:N - sh]).sum()
        colsum[sh:] += gc[:N - sh]
        msesum += float(ua[:, NSUB + NSLOT].sum())

    # tie correction: equal-label pairs must contribute term M, grad 0
    vals, starts, counts = np.unique(ls, return_index=True, return_counts=True)
    for s, cnt in zip(starts, counts):
        if cnt > 1:
            for a in range(s, s + cnt):
                for b in range(a + 1, s + cnt):
                    z = MARGIN + q[a] - q[b]
                    t_total += MARGIN - max(z, 0.0)
                    if z > 0:
                        rowsum[a] -= 1.0
                        colsum[b] -= 1.0

    g_vec = rowsum - colsum
    g2 = np.sqrt((g_vec * g_vec).sum())
    mse = msesum / N
    g1 = 2.0 * np.sqrt(msesum) / N
    return np.float32(mse + (g1 / (g2 + EPS)) * t_total)


def kernel(preds, labels):
    global LAST_RESULTS
    from concourse.bass_utils import run_bass_kernel_spmd

    if "nc" not in _CACHE:
        _CACHE["nc"] = build_nc()
    in_maps = make_in_maps(preds, labels)
    res = run_bass_kernel_spmd(_CACHE["nc"], in_maps, list(range(NCORES)))
    LAST_RESULTS = res
    return combine(res.results, preds, labels)
